# revision 14
# baseline (speedup 1.0000x reference)
"""Trainium2 Bass kernel for nn_DynamLinear: per-codebook linear -> chunked
outer product -> mean over codebooks -> RMS norm.

Math notes:
  ref: y = einsum('td,hdo->tho', x, W); split o=64 into a=y[..., :32], b=y[..., 32:]
       op[t,h,i,j] = a[t,h,i]*b[t,h,j];  out = mean_h(op)*sqrt(16); rms_norm(out)
  Since rms_norm is scale invariant, out = S / sqrt(mean(S^2) + 16e-12) where
       S[t,i,j] = sum_h a[t,h,i]*b[t,h,j]  (per-token 16x32^T @ 16x32 matmul)

Per-core plan (tokens sharded 1024/core):
  stage1: y^T = Wp^T @ x^T on TensorE (bf16), col = 512*sel + 128*ctp + 16*m + h
          (i or j = 8*ctp + m), PSUM -> y_sb via ACT copies.
  shuffle: y_sb -> z4[32*q+h, sel, ctp, m, t256] via per-(q, sel, m) DMAs
          (790ns engine-busy each in the cost model), distributed across
          Pool, ACT, and SP so no single engine eats the 50us aggregate.
  stage2: per token one matmul lhsT=A_t[16h x 32i], rhs=B_t[16h x 32j] on a
          32x32 PE tile (row group q, col group c = token%4) -> PSUM
          S[32c+i, t32, j] per 64-token chunk.
  rms:    ACT square (bf16), DVE reduce over j, one indicator-matmul per
          quarter reduces over i and broadcasts, ACT sqrt + DVE reciprocal,
          DVE multiply (bf16 out), paired DMA store.  ind-matmuls are
          deferred ~a quarter so the PE never stalls on the DVE reduces.
  A train of dummy PE matmuls during the input loads ramps the PE to max
  pstate before stage-1 begins.
"""

import sys
import functools
from contextlib import ExitStack

import numpy as np
import ml_dtypes

if "/opt/trn_rl_repo" not in sys.path:
    sys.path.insert(0, "/opt/trn_rl_repo")

import concourse.bass as bass
import concourse.bacc as bacc
import concourse.tile as tile
from concourse import mybir
from concourse.bass_utils import run_bass_kernel_spmd

N_CORES = 8
T_CORE = 1024          # tokens per core
D = 1024               # feat dim
H = 16                 # codebooks
EPS = 16e-12           # 16 * 1e-12 (scale-folded reference eps)
N_WARM = 64            # PE warm-up matmuls during input DMA

F32 = mybir.dt.float32
F32R = mybir.dt.float32r
BF16 = mybir.dt.bfloat16


def _kernel_body(tc, ctx, xt, wp, out):
    nc = tc.nc

    singles = ctx.enter_context(tc.tile_pool(name="singles", bufs=1))
    ps1 = ctx.enter_context(tc.tile_pool(name="ps1", bufs=2, space="PSUM"))
    ps2p = ctx.enter_context(tc.tile_pool(name="ps2p", bufs=6, space="PSUM"))
    scr = ctx.enter_context(tc.tile_pool(name="scr", bufs=3))
    smalls = ctx.enter_context(tc.tile_pool(name="smalls", bufs=4))

    # ---- resident tiles --------------------------------------------------
    wp_sb = singles.tile([128, 8, 8, 128], BF16)   # [dp, CT, dt, colp]
    xt_sb = singles.tile([128, 8, 1024], BF16)     # [dp, dt, t]
    y_sb = singles.tile([128, 8, 1024], BF16)      # [16m+h, CT, t]
    z4 = singles.tile([128, 2, 4, 8, 256], BF16)   # [32q+h, sel, ctp, m, t256]
    zout = singles.tile([128, 16, 16, 32], BF16)   # [32c+i, chunk, t32, j]
    ind_sb = singles.tile([128, 128], BF16)        # block-diag ones (4x 32x32)
    eps_sb = singles.tile([128, 1], F32)
    dummy = singles.tile([128, 128], BF16)         # PE warm-up operand

    # small constants (DVE, idle at start)
    nc.vector.memset(eps_sb[:], EPS)
    nc.vector.memset(dummy[:], 0.0)
    nc.vector.memset(ind_sb[:], 0.0)
    for c in range(4):
        nc.vector.memset(ind_sb[32 * c:32 * c + 32, 32 * c:32 * c + 32], 1.0)

    # ---- input DMAs: wp + xt q0 on SP, xt q1-3 on ACT (idle early) -------
    nc.sync.dma_start(out=wp_sb[:, 0:2], in_=wp[:, 0:2])
    nc.scalar.dma_start(out=xt_sb[:, 0:4, 0:256], in_=xt[:, 0:4, 0:256])
    nc.scalar.dma_start(out=xt_sb[:, 4:8, 0:256], in_=xt[:, 4:8, 0:256])
    nc.sync.dma_start(out=wp_sb[:, 2:4], in_=wp[:, 2:4])
    nc.gpsimd.dma_start(out=wp_sb[:, 4:6], in_=wp[:, 4:6])
    nc.gpsimd.dma_start(out=wp_sb[:, 6:8], in_=wp[:, 6:8])
    for q in range(1, 4):
        nc.scalar.dma_start(out=xt_sb[:, :, q * 256:(q + 1) * 256],
                            in_=xt[:, :, q * 256:(q + 1) * 256])

    # ---- PE warm-up: ramp to max pstate while inputs load ----------------
    ps_warm = ps1.tile([128, 4, 16], F32, name="ps_warm", tag="psb")
    warm_flat = ps_warm.rearrange("p a b -> p (a b)")
    for _ in range(N_WARM):
        nc.tensor.matmul(warm_flat[:], lhsT=dummy[:], rhs=dummy[:, 0:64],
                         start=True, stop=True)

    out4 = out.rearrange("p (pair f) -> p pair f", pair=8)

    def _s1_group(q, g):
        t0 = q * 256
        psA = ps1.tile([128, 2, 256], F32, name="psA", tag="psb")
        for k in range(2):
            CT = 2 * g + k
            for d in range(8):
                nc.tensor.matmul(
                    psA[:, k, :],
                    lhsT=wp_sb[:, CT, d, :],
                    rhs=xt_sb[:, d, t0:t0 + 256],
                    start=(d == 0),
                    stop=(d == 7),
                )
        nc.scalar.activation(
            y_sb[:, 2 * g:2 * g + 2, t0:t0 + 256], psA[:],
            mybir.ActivationFunctionType.Copy,
        )

    # engine plan per 8-unit batch: Pool x3, SP x3, Act x2
    SHUF_ENGS = ["p", "s", "p", "s", "a", "p", "s", "a"]

    def _shuffle_units(q, g):
        # shuffle batches: coarse 790ns units per sel (after its 2nd copy);
        # for the critical last quarter's sel1, fine 500ns-floor units per
        # copy-group so they start a stage-1 group earlier
        t0 = q * 256
        emap = {"p": nc.gpsimd, "s": nc.sync, "a": nc.scalar}
        if q == 3 and g >= 2:
            gp = g % 2
            for m in range(8):
                emap[SHUF_ENGS[m]].dma_start(
                    out=z4[32 * q:32 * q + 16, 1, 2 * gp:2 * gp + 2, m, :],
                    in_=y_sb[16 * m:16 * m + 16, 2 * g:2 * g + 2,
                             t0:t0 + 256],
                )
        elif g % 2 == 1:
            sel = g // 2
            for m in range(8):
                emap[SHUF_ENGS[m]].dma_start(
                    out=z4[32 * q:32 * q + 16, sel, :, m, :],
                    in_=y_sb[16 * m:16 * m + 16, 4 * sel:4 * sel + 4,
                             t0:t0 + 256],
                )

    # per-quarter state carried to the deferred norm pass
    ps2_tiles = [None] * 16
    part_tiles = [None] * 4

    def _s2_chunk(ch):
        q, half = ch // 4, ch % 4
        if part_tiles[q] is None:
            part_tiles[q] = smalls.tile([128, 4, 16], BF16, name="part", tag="part")
        part_q = part_tiles[q]
        t0 = 64 * half
        ps2 = ps2p.tile([128, 16, 32], F32, name="ps2", tag="ps2")
        ps2_tiles[ch] = ps2
        for tw in range(64):
            c, t32 = tw % 4, tw // 4
            t256 = t0 + tw
            nc.tensor.matmul(
                ps2[32 * c:32 * c + 32, t32, :],
                lhsT=z4[32 * q:32 * q + 16, 0, :, :, t256],
                rhs=z4[32 * q:32 * q + 16, 1, :, :, t256],
                start=True, stop=True,
                tile_position=(32 * q, 32 * c),
            )
        sq = scr.tile([128, 16, 32], BF16, name="sq", tag="sq")
        nc.scalar.square(sq[:], ps2[:])
        with nc.allow_low_precision(reason="bf16 sum of 32 sq for rms"):
            nc.vector.tensor_reduce(part_q[:, half], sq[:],
                                    axis=mybir.AxisListType.X,
                                    op=mybir.AluOpType.add)

    def _norm(q, halves):
        # ind-matmul reduces over i (and broadcasts); emitted only once the
        # DVE reduces for these halves are safely complete
        part_q = part_tiles[q]
        nh = len(halves)
        h0 = halves[0]
        ps3 = ps1.tile([128, nh, 16], F32, name="ps3", tag="psb")
        nc.tensor.matmul(ps3.rearrange("p a b -> p (a b)"),
                         lhsT=ind_sb[:],
                         rhs=part_q[:, h0:h0 + nh].rearrange(
                             "p a b -> p (a b)"),
                         start=True, stop=True)
        s_sb = smalls.tile([128, nh, 16], F32, name="s_sb", tag="s_sb")
        nc.scalar.activation(s_sb[:], ps3[:],
                             mybir.ActivationFunctionType.Sqrt,
                             bias=eps_sb[:], scale=1.0 / 1024.0)
        rstd = smalls.tile([128, nh, 16], F32, name="rstd", tag="rstd")
        nc.vector.reciprocal(rstd[:], s_sb[:])
        for u, half in enumerate(halves):
            ch = 4 * q + half
            nc.vector.tensor_mul(
                zout[:, ch], ps2_tiles[ch][:],
                rstd[:, u].unsqueeze(2).broadcast_to([128, 16, 32]))
            ps2_tiles[ch] = None
        if q == 3 and halves == [2, 3]:
            out16 = out.rearrange("p (ch f) -> p ch f", ch=16)
            nc.sync.dma_start(out=out16[:, 14],
                              in_=zout[:, 14].rearrange("p a b -> p (a b)"))
            nc.scalar.dma_start(out=out16[:, 15],
                                in_=zout[:, 15].rearrange("p a b -> p (a b)"))
        else:
            for pair in range((4 * q + halves[0]) // 2,
                              (4 * q + halves[-1] + 1) // 2):
                nc.gpsimd.dma_start(
                    out=out4[:, pair],
                    in_=zout[:, 2 * pair:2 * pair + 2].rearrange(
                        "p a b c -> p (a b c)"),
                )

    def _stage1(q, norm_q=None):
        _s1_group(q, 0)
        if norm_q is not None:
            _norm(norm_q, [0, 1, 2, 3])
        _s1_group(q, 1)
        _shuffle_units(q, 1)
        _s1_group(q, 2)
        _shuffle_units(q, 2)
        _s1_group(q, 3)
        _shuffle_units(q, 3)

    _stage1(0)
    _stage1(1)
    for ch in range(4):
        _s2_chunk(ch)
    _stage1(2, norm_q=0)
    for ch in range(4, 8):
        _s2_chunk(ch)
    _stage1(3, norm_q=1)
    for ch in range(8, 12):
        _s2_chunk(ch)
    _s2_chunk(12)
    _s2_chunk(13)
    _norm(2, [0, 1, 2, 3])
    _s2_chunk(14)
    _s2_chunk(15)
    _norm(3, [0, 1])
    _norm(3, [2, 3])


@functools.lru_cache(maxsize=1)
def _build_program():
    nc = bacc.Bacc("TRN2", target_bir_lowering=False, debug=False)
    xt = nc.dram_tensor("xt", [128, 8, 1024], BF16, kind="ExternalInput").ap()
    wp = nc.dram_tensor("wp", [128, 8, 8, 128], BF16, kind="ExternalInput").ap()
    out = nc.dram_tensor("out", [128, 8192], BF16, kind="ExternalOutput").ap()
    with tile.TileContext(nc) as tc:
        with ExitStack() as ctx:
            _kernel_body(tc, ctx, xt, wp, out)
    nc.compile()
    return nc


def _host_prep(x, weight):
    xf = np.ascontiguousarray(x.reshape(-1, D))           # [8192, 1024]
    # col = 512*sel + 128*ctp + 16*m + h ; i or j = 8*ctp + m
    w = weight.transpose(1, 0, 2).reshape(D, H, 2, 4, 8)  # [d, h, sel, ctp, m]
    wpm = w.transpose(0, 2, 3, 4, 1).reshape(D, 1024)     # [d, col]
    wp_sb = np.ascontiguousarray(
        wpm.reshape(8, 128, 8, 128).transpose(1, 2, 0, 3)).astype(
            ml_dtypes.bfloat16)                           # [dp, CT, dt, colp]
    xt_shards = []
    for c in range(N_CORES):
        xtc = xf[c * T_CORE:(c + 1) * T_CORE].T           # [d, t]
        xt_sb = np.ascontiguousarray(
            xtc.reshape(8, 128, 1024).transpose(1, 0, 2)).astype(
                ml_dtypes.bfloat16)
        xt_shards.append(xt_sb)
    return xt_shards, wp_sb


def kernel(x, weight, **_unused):
    x = np.asarray(x, dtype=np.float32)
    weight = np.asarray(weight, dtype=np.float32)
    xt_shards, wp_sb = _host_prep(x, weight)
    nc = _build_program()
    in_maps = [{"xt": xt_shards[c], "wp": wp_sb} for c in range(N_CORES)]
    res = run_bass_kernel_spmd(nc, in_maps, list(range(N_CORES)))
    outs = []
    for c in range(N_CORES):
        d = np.asarray(res.results[c]["out"]).astype(np.float32)
        d = d.reshape(4, 32, 16, 16, 32)
        # [cg, i, ch, t32, j] -> token t = 64*ch + 4*t32 + cg, feat = 32*i+j
        outs.append(d.transpose(2, 3, 0, 1, 4).reshape(T_CORE, 1024))
    full = np.concatenate(outs, axis=0)                   # [8192, 1024]
    return full.reshape(x.shape[0], x.shape[1], 1024).astype(np.float32)


if __name__ == "__main__":
    rng = np.random.default_rng(0)
    x = rng.standard_normal((4, 2048, D), dtype=np.float32)
    w = (rng.standard_normal((H, D, 64), dtype=np.float32)
         * np.sqrt(2.0 / (D + 64))).astype(np.float32)
    o = kernel(x, w)
    print(o.shape, o.dtype)
